# revision 31
# baseline (speedup 1.0000x reference)
"""Trainium2 Bass kernel for nn_Embed_38766374814290 (embedding_lookup).

Math: out[i,j,l,e] = A[m][e] + delta_s[i,j,l] * B[m][e]
  where m = (j < traj_len[i]), delta_s = where(m, mat2[traj_loc-1], 0),
  A[m] = emb_sl_w[m] + emb_tl_w[m],
  B[m] = (emb_su_w[m]-emb_sl_w[m])/SU + (emb_tu_w[m]-emb_tl_w[m])/TU.

Sharding: pure data parallel over batch N = 32 -> 4 rows per core x 8 cores.

The kernel is HBM-write-bound, so everything is fp16 end to end (the
2e-2 rel-err gate leaves ~27x margin at fp16; host upcasts to f32):

  1. indirect-gather G[pos, l] = mat2x[idx[pos], l] in fp16 (idx
     redirects invalid positions to an appended all-zero row 4096).
  2. Four PE transposes pack G l-slices into one [32, 512] PSUM tile;
     one DVE copy evicts it to SBUF (rows 0-31 of the lhsT tile); a
     tiny DMA writes 2 constant rows [m, 1] (rows 32-33).
  3. Sixteen K=34 fp16 matmuls per row against constant block-diagonal
     rhs build out[pos, (l,e)] = G*b1 + m*dA + a0 in f32 PSUM.
  4. PSUM -> SBUF evictions convert f32 -> fp16 in [128, 1024] chunks
     spanning 2 PSUM banks, alternating between ACT and DVE.
  5. fp16 output rows DMA out in 512 KiB fully-contiguous-per-partition
     chunks (4 KiB per partition), halving the write traffic vs f32.
"""
import os
import numpy as np
from contextlib import ExitStack

SU, TU = 10000.0, 86400.0
N, M, L, E = 32, 128, 128, 64
NLOC = 4096
NCORES = 8
ROWS = N // NCORES  # 4 batch rows per core
K = 34              # 32 G^T rows + [m, 1]

_CACHE = {}


def _install_profhook():
    """Optional: shim the missing antenv.axon_hooks so trace=True works."""
    import sys
    import types
    if "antenv.axon_hooks" in sys.modules:
        return True
    try:
        from trn_agent_boot.trn_boot import _ntff_profile_via_ctypes
    except Exception:
        return False
    hook = [None]
    mod = types.ModuleType("antenv.axon_hooks")
    mod.set_axon_ntff_profile_hook = lambda h: hook.__setitem__(0, h)
    mod.get_axon_ntff_profile_hook = lambda: hook[0]
    sys.modules["antenv.axon_hooks"] = mod
    try:
        mod.set_axon_ntff_profile_hook(
            _ntff_profile_via_ctypes("/opt/axon/libaxon_pjrt.so"))
    except Exception:
        return False
    return True


def _build():
    import concourse.bass as bass
    import concourse.tile as tile
    from concourse import bacc, mybir

    F32 = mybir.dt.float32
    F16 = mybir.dt.float16
    I32 = mybir.dt.int32

    nc = bacc.Bacc("TRN2", target_bir_lowering=False, debug=False,
                   enable_asserts=True, num_devices=NCORES)
    m2_d = nc.dram_tensor("m2", [NLOC + 1, L], F16, kind="ExternalInput").ap()
    idx_d = nc.dram_tensor("idx", [M, ROWS], I32, kind="ExternalInput").ap()
    mrow_d = nc.dram_tensor("mrow", [ROWS, 2, 4 * M], F16,
                            kind="ExternalInput").ap()
    rhs_d = nc.dram_tensor("rhs", [4, K, 8 * E], F16,
                           kind="ExternalInput").ap()
    ident_d = nc.dram_tensor("ident", [128, 128], F16,
                             kind="ExternalInput").ap()
    out_d = nc.dram_tensor("out", [ROWS, M, L * E], F16,
                           kind="ExternalOutput").ap()

    with tile.TileContext(nc) as tc, ExitStack() as ctx:
        const = ctx.enter_context(tc.tile_pool(name="const", bufs=1))
        ipool = ctx.enter_context(tc.tile_pool(name="idxp", bufs=1))
        gpool = ctx.enter_context(tc.tile_pool(name="gath", bufs=1))
        gtpool = ctx.enter_context(tc.tile_pool(name="gt", bufs=1))
        opool = ctx.enter_context(tc.tile_pool(name="orow", bufs=2))
        pso = ctx.enter_context(tc.tile_pool(name="pso", bufs=4, space="PSUM"))

        # Small latency-critical DMAs first on the sync (HWDGE) queue: idx
        # heads the critical path (idx -> indirect gather -> transpose ->
        # matmul -> evict -> out-DMA), and the mask rows gate the first
        # matmul of each row.  Bulk consts (ident, rhs) go on the scalar
        # queue so they don't delay these.
        it = ipool.tile([M, ROWS], I32)
        nc.sync.dma_start(it[:], idx_d[:])
        # Persistent per-row lhsT tiles; mask rows 32-33 DMA up-front so no
        # HBM DMA sits inside the row loop (a row-loop DMA on an eviction
        # engine's queue would serialize row-to-row).
        gtrows = []
        for i in range(ROWS):
            gt = gtpool.tile([K, 8 * E], F16, tag=f"gt{i}")
            nc.sync.dma_start(gt[32:34, :], mrow_d[i])
            gtrows.append(gt)
        ident = const.tile([128, 128], F16)
        nc.scalar.dma_start(ident[:], ident_d[:])
        # HAM warmup: ~5us of back-to-back cold matmuls at t=0 lifts the
        # PE clock gate to 8/8 before the first gather lands; the real
        # burst then runs at 2.4 GHz. Results are never read.
        # PSUM plan: 4 x [128,1024] f32 pair-tiles = all 8 banks.  Each
        # eviction is one [128,1024] two-bank copy (half the fixed cost of
        # per-bank evictions keeps ACT+DVE under the DMA write floor).
        # The per-row transpose staging lives INSIDE the ring: transposes
        # use an fp16 bitcast view of the row's first pair-tile, evicted
        # to gtrow before that pair's matmuls overwrite it.
        # No HAM warmup: the clock gate is flaky (observed stuck cold for
        # 27us of 100% PE busy, and re-tripped by the stall bursts a warm
        # PE necessarily hits when it outruns the eviction/DMA drain).
        # Starting the real stream ~6us earlier beats chasing 2.4 GHz:
        # cold-paced the phase is 27us; if HAM does unthrottle mid-stream
        # the drain absorbs the speedup automatically.
        rhs_tiles = []
        for s in range(4):
            rt = const.tile([K, 8 * E], F16, tag=f"rhs{s}")
            nc.scalar.dma_start(rt[:], rhs_d[s])
            rhs_tiles.append(rt)

        gs = []
        for i in range(ROWS):
            g = gpool.tile([128, L], F16, tag=f"g{i}")
            nc.gpsimd.indirect_dma_start(
                out=g[:], out_offset=None, in_=m2_d[:],
                in_offset=bass.IndirectOffsetOnAxis(ap=it[:, i:i + 1], axis=0))
            gs.append(g)

        def emit_head(i):
            # Transposes + gtrow fill for row i, staged in an fp16 view of
            # the row's first pair-tile.  Called mid-stream of the previous
            # row so the PE never idles across row boundaries (long PE gaps
            # trip the HAM clock gate back to 1.2 GHz).
            po0 = pso.tile([128, 1024], F32, tag="po", name=f"poh{i}")
            ptv = po0.bitcast(F16)
            for q in range(4):
                nc.tensor.transpose(out=ptv[0:32, 128 * q:128 * (q + 1)],
                                    in_=gs[i][:, 32 * q:32 * (q + 1)],
                                    identity=ident[:])
            nc.scalar.copy(out=gtrows[i][0:32, :], in_=ptv[0:32, 0:512])
            return po0

        heads = [None] * ROWS
        heads[0] = emit_head(0)
        pair = 0
        for i in range(ROWS):
            orow = opool.tile([128, L * E], F16)
            for pr in range(8):
                gi = pr // 2
                po = heads[i] if pr == 0 else pso.tile(
                    [128, 1024], F32, tag="po", name=f"po{i}_{pr}")
                for h in range(2):
                    s = (pr % 2) * 2 + h
                    nc.tensor.matmul(po[:, 512 * h:512 * (h + 1)],
                                     lhsT=gtrows[i][:, 128 * gi:128 * (gi + 1)],
                                     rhs=rhs_tiles[s][:],
                                     start=True, stop=True)
                if pr == 3 and i + 1 < ROWS:
                    # pr==3: the head's ACT copy then queues AFTER evict-2
                    # (at pr==2 it sat between evict-0 and evict-2 and the
                    # delayed evict-2 stalled the PE at pair 5's ring WAR);
                    # by evict-4 the ring has slack. Still early enough to
                    # clear before the row boundary.
                    heads[i + 1] = emit_head(i + 1)
                lo = 2048 * gi + 1024 * (pr % 2)
                dst = orow[:, lo:lo + 1024]
                # Every pair-eviction splits across BOTH engines (ACT cols
                # 0:512 = bank A, DVE 512:1024 = bank B — parallel PSUM
                # reads on different banks are allowed).  Eviction latency
                # drops ~1.2us -> ~0.7us, giving the ring WAR the slack
                # that removes the per-row PE stalls at cold pace; both
                # engines stay under the cold PE stream time.
                nc.scalar.copy(out=dst[:, 0:512], in_=po[:, 0:512])
                nc.vector.tensor_copy(out=dst[:, 512:1024],
                                      in_=po[:, 512:1024])
                pair += 1
                if (i == 0 and gi == 0) or (i == ROWS - 1 and pr >= 6):
                    # First window: per-pair DMAs start the write phase
                    # earlier.  Last pairs: per-half DMAs drain the pure
                    # critical-path tail sooner.
                    if i == ROWS - 1 and pr >= 6:
                        nc.sync.dma_start(out_d[i][:, lo:lo + 512],
                                          orow[:, lo:lo + 512])
                        nc.sync.dma_start(out_d[i][:, lo + 512:lo + 1024],
                                          orow[:, lo + 512:lo + 1024])
                    else:
                        nc.sync.dma_start(out_d[i][:, lo:lo + 1024],
                                          orow[:, lo:lo + 1024])
                elif pr % 2 == 1:
                    nc.sync.dma_start(out_d[i][:, 2048 * gi:2048 * (gi + 1)],
                                      orow[:, 2048 * gi:2048 * (gi + 1)])
    nc.compile()
    return nc


def kernel(traj_loc, mat2, vec, traj_len, l_max, emb_sl_w, emb_su_w,
           emb_tl_w, emb_tu_w):
    from concourse import bass_utils

    traj_loc = np.asarray(traj_loc).astype(np.int64)
    mat2 = np.ascontiguousarray(np.asarray(mat2, dtype=np.float32))
    traj_len = np.asarray(traj_len).astype(np.int64)
    esl = np.asarray(emb_sl_w, dtype=np.float32)
    esu = np.asarray(emb_su_w, dtype=np.float32)
    etl = np.asarray(emb_tl_w, dtype=np.float32)
    etu = np.asarray(emb_tu_w, dtype=np.float32)

    # host prep: constants
    A = esl + etl                                            # [2, E]
    B = (esu - esl) / np.float32(SU) + (etu - etl) / np.float32(TU)
    mask = (np.arange(M)[None, :] < traj_len[:, None])       # [N, M]
    idx_full = np.where(mask, traj_loc - 1, NLOC).astype(np.int32)

    mat2x = np.concatenate([mat2, np.zeros((1, L), np.float32)], axis=0)
    m2 = mat2x.astype(np.float16)
    b1 = B[1].astype(np.float16)
    dA = (A[1] - A[0]).astype(np.float16)
    a0 = A[0].astype(np.float16)

    # rhs[s] is [34, 8E]: row 8*s+lp selects l' = lp within the window and
    # scales e-block lp by b1; rows 32-33 pair with lhsT rows [m, 1]:
    # m*dA + a0, replicated across all 8 e-blocks.
    rhs = np.zeros((4, K, 8 * E), np.float16)
    for s in range(4):
        for lp in range(8):
            rhs[s, 8 * s + lp, E * lp:E * (lp + 1)] = b1
        rhs[s, 32, :] = np.tile(dA, 8)
        rhs[s, 33, :] = np.tile(a0, 8)
    ident = np.eye(128, dtype=np.float16)

    # mrow[i] = [m, 1] rows for lhsT rows 32-33, tiled 4x along the free
    # dim so one DMA fills all four gi windows of a row's lhsT tile.
    mrow_full = np.empty((N, 2, 4 * M), np.float16)
    mrow_full[:, 0, :] = np.tile(mask.astype(np.float16), (1, 4))
    mrow_full[:, 1, :] = np.ones((1, 4 * M), np.float16)

    if "nc" not in _CACHE:
        _CACHE["nc"] = _build()
    nc = _CACHE["nc"]

    in_maps = []
    for c in range(NCORES):
        sl = slice(ROWS * c, ROWS * (c + 1))
        in_maps.append({
            "m2": m2,
            "idx": np.ascontiguousarray(idx_full[sl].T),
            "mrow": np.ascontiguousarray(mrow_full[sl]),
            "rhs": rhs,
            "ident": ident,
        })

    trace = os.environ.get("KERNEL_TRACE", "0") == "1" and _install_profhook()
    res = bass_utils.run_bass_kernel_spmd(
        nc, in_maps, core_ids=list(range(NCORES)), trace=bool(trace))
    if trace:
        _CACHE["exec_time_ns"] = res.exec_time_ns
        _CACHE["trace_path"] = (res.instructions_and_trace or (None, None))[1]
        _CACHE["tmpdir"] = res.profile_json

    out = np.concatenate(
        [res.results[c]["out"].reshape(ROWS, M, L, E) for c in range(NCORES)],
        axis=0)
    return out.astype(np.float32)


# revision 32
# speedup vs baseline: 1.0112x; 1.0112x over previous
"""Trainium2 Bass kernel for nn_Embed_38766374814290 (embedding_lookup).

Math: out[i,j,l,e] = A[m][e] + delta_s[i,j,l] * B[m][e]
  where m = (j < traj_len[i]), delta_s = where(m, mat2[traj_loc-1], 0),
  A[m] = emb_sl_w[m] + emb_tl_w[m],
  B[m] = (emb_su_w[m]-emb_sl_w[m])/SU + (emb_tu_w[m]-emb_tl_w[m])/TU.

Sharding: pure data parallel over batch N = 32 -> 4 rows per core x 8 cores.

The kernel is HBM-write-bound, so everything is fp16 end to end (the
2e-2 rel-err gate leaves ~27x margin at fp16; host upcasts to f32):

  1. indirect-gather G[pos, l] = mat2x[idx[pos], l] in fp16 (idx
     redirects invalid positions to an appended all-zero row 4096).
  2. Four PE transposes pack G l-slices into one [32, 512] PSUM tile;
     one DVE copy evicts it to SBUF (rows 0-31 of the lhsT tile); a
     tiny DMA writes 2 constant rows [m, 1] (rows 32-33).
  3. Sixteen K=34 fp16 matmuls per row against constant block-diagonal
     rhs build out[pos, (l,e)] = G*b1 + m*dA + a0 in f32 PSUM.
  4. PSUM -> SBUF evictions convert f32 -> fp16 in [128, 1024] chunks
     spanning 2 PSUM banks, alternating between ACT and DVE.
  5. fp16 output rows DMA out in 512 KiB fully-contiguous-per-partition
     chunks (4 KiB per partition), halving the write traffic vs f32.
"""
import os
import numpy as np
from contextlib import ExitStack

SU, TU = 10000.0, 86400.0
N, M, L, E = 32, 128, 128, 64
NLOC = 4096
NCORES = 8
ROWS = N // NCORES  # 4 batch rows per core
K = 34              # 32 G^T rows + [m, 1]

_CACHE = {}


def _install_profhook():
    """Optional: shim the missing antenv.axon_hooks so trace=True works."""
    import sys
    import types
    if "antenv.axon_hooks" in sys.modules:
        return True
    try:
        from trn_agent_boot.trn_boot import _ntff_profile_via_ctypes
    except Exception:
        return False
    hook = [None]
    mod = types.ModuleType("antenv.axon_hooks")
    mod.set_axon_ntff_profile_hook = lambda h: hook.__setitem__(0, h)
    mod.get_axon_ntff_profile_hook = lambda: hook[0]
    sys.modules["antenv.axon_hooks"] = mod
    try:
        mod.set_axon_ntff_profile_hook(
            _ntff_profile_via_ctypes("/opt/axon/libaxon_pjrt.so"))
    except Exception:
        return False
    return True


def _build():
    import concourse.bass as bass
    import concourse.tile as tile
    from concourse import bacc, mybir

    F32 = mybir.dt.float32
    F16 = mybir.dt.float16
    I32 = mybir.dt.int32

    nc = bacc.Bacc("TRN2", target_bir_lowering=False, debug=False,
                   enable_asserts=True, num_devices=NCORES)
    m2_d = nc.dram_tensor("m2", [NLOC + 1, L], F16, kind="ExternalInput").ap()
    idx_d = nc.dram_tensor("idx", [M, ROWS], I32, kind="ExternalInput").ap()
    mrow_d = nc.dram_tensor("mrow", [ROWS, 2, 4 * M], F16,
                            kind="ExternalInput").ap()
    rhs_d = nc.dram_tensor("rhs", [4, K, 8 * E], F16,
                           kind="ExternalInput").ap()
    ident_d = nc.dram_tensor("ident", [128, 128], F16,
                             kind="ExternalInput").ap()
    out_d = nc.dram_tensor("out", [ROWS, M, L * E], F16,
                           kind="ExternalOutput").ap()

    with tile.TileContext(nc) as tc, ExitStack() as ctx:
        const = ctx.enter_context(tc.tile_pool(name="const", bufs=1))
        ipool = ctx.enter_context(tc.tile_pool(name="idxp", bufs=1))
        gpool = ctx.enter_context(tc.tile_pool(name="gath", bufs=1))
        gtpool = ctx.enter_context(tc.tile_pool(name="gt", bufs=1))
        opool = ctx.enter_context(tc.tile_pool(name="orow", bufs=2))
        pso = ctx.enter_context(tc.tile_pool(name="pso", bufs=4, space="PSUM"))

        # Small latency-critical DMAs first on the sync (HWDGE) queue: idx
        # heads the critical path (idx -> indirect gather -> transpose ->
        # matmul -> evict -> out-DMA), and the mask rows gate the first
        # matmul of each row.  Bulk consts (ident, rhs) go on the scalar
        # queue so they don't delay these.
        it = ipool.tile([M, ROWS], I32)
        nc.sync.dma_start(it[:], idx_d[:])
        # Persistent per-row lhsT tiles; mask rows 32-33 DMA up-front so no
        # HBM DMA sits inside the row loop (a row-loop DMA on an eviction
        # engine's queue would serialize row-to-row).
        gtrows = []
        for i in range(ROWS):
            gt = gtpool.tile([K, 8 * E], F16, tag=f"gt{i}")
            nc.sync.dma_start(gt[32:34, :], mrow_d[i])
            gtrows.append(gt)
        ident = const.tile([128, 128], F16)
        nc.scalar.dma_start(ident[:], ident_d[:])
        # HAM warmup: ~5us of back-to-back cold matmuls at t=0 lifts the
        # PE clock gate to 8/8 before the first gather lands; the real
        # burst then runs at 2.4 GHz. Results are never read.
        # PSUM plan: 4 x [128,1024] f32 pair-tiles = all 8 banks.  Each
        # eviction is one [128,1024] two-bank copy (half the fixed cost of
        # per-bank evictions keeps ACT+DVE under the DMA write floor).
        # The per-row transpose staging lives INSIDE the ring: transposes
        # use an fp16 bitcast view of the row's first pair-tile, evicted
        # to gtrow before that pair's matmuls overwrite it.
        # No HAM warmup: the clock gate is flaky (observed stuck cold for
        # 27us of 100% PE busy, and re-tripped by the stall bursts a warm
        # PE necessarily hits when it outruns the eviction/DMA drain).
        # Starting the real stream ~6us earlier beats chasing 2.4 GHz:
        # cold-paced the phase is 27us; if HAM does unthrottle mid-stream
        # the drain absorbs the speedup automatically.
        rhs_tiles = []
        for s in range(4):
            rt = const.tile([K, 8 * E], F16, tag=f"rhs{s}")
            nc.scalar.dma_start(rt[:], rhs_d[s])
            rhs_tiles.append(rt)

        gs = []
        for i in range(ROWS):
            g = gpool.tile([128, L], F16, tag=f"g{i}")
            nc.gpsimd.indirect_dma_start(
                out=g[:], out_offset=None, in_=m2_d[:],
                in_offset=bass.IndirectOffsetOnAxis(ap=it[:, i:i + 1], axis=0))
            gs.append(g)

        def emit_head(i):
            # Transposes + gtrow fill for row i, staged in an fp16 view of
            # the row's first pair-tile.  Called mid-stream of the previous
            # row so the PE never idles across row boundaries (long PE gaps
            # trip the HAM clock gate back to 1.2 GHz).
            po0 = pso.tile([128, 1024], F32, tag="po", name=f"poh{i}")
            ptv = po0.bitcast(F16)
            for q in range(4):
                nc.tensor.transpose(out=ptv[0:32, 128 * q:128 * (q + 1)],
                                    in_=gs[i][:, 32 * q:32 * (q + 1)],
                                    identity=ident[:])
            nc.scalar.copy(out=gtrows[i][0:32, :], in_=ptv[0:32, 0:512])
            return po0

        heads = [None] * ROWS
        heads[0] = emit_head(0)
        pair = 0
        for i in range(ROWS):
            orow = opool.tile([128, L * E], F16)
            for pr in range(8):
                gi = pr // 2
                po = heads[i] if pr == 0 else pso.tile(
                    [128, 1024], F32, tag="po", name=f"po{i}_{pr}")
                for h in range(2):
                    s = (pr % 2) * 2 + h
                    nc.tensor.matmul(po[:, 512 * h:512 * (h + 1)],
                                     lhsT=gtrows[i][:, 128 * gi:128 * (gi + 1)],
                                     rhs=rhs_tiles[s][:],
                                     start=True, stop=True)
                if pr == 3 and i + 1 < ROWS:
                    # pr==3: the head's ACT copy then queues AFTER evict-2
                    # (at pr==2 it sat between evict-0 and evict-2 and the
                    # delayed evict-2 stalled the PE at pair 5's ring WAR);
                    # by evict-4 the ring has slack. Still early enough to
                    # clear before the row boundary.
                    heads[i + 1] = emit_head(i + 1)
                lo = 2048 * gi + 1024 * (pr % 2)
                dst = orow[:, lo:lo + 1024]
                if i == ROWS - 1 and pr >= 6:
                    # Tail: the last pairs' evict + DMA are pure critical
                    # path after the PE stream ends.  Split each eviction
                    # across both engines ([128,512] halves concurrently)
                    # and DMA per half.
                    nc.scalar.copy(out=dst[:, 0:512], in_=po[:, 0:512])
                    nc.vector.tensor_copy(out=dst[:, 512:1024],
                                          in_=po[:, 512:1024])
                    nc.sync.dma_start(out_d[i][:, lo:lo + 512],
                                      orow[:, lo:lo + 512])
                    nc.sync.dma_start(out_d[i][:, lo + 512:lo + 1024],
                                      orow[:, lo + 512:lo + 1024])
                    pair += 1
                    continue
                # Strict ACT/DVE alternation: consecutive same-engine
                # evictions serialize and stall the PE on the ring WAR.
                # Head copies slot into ACT's idle gap after pr==2.
                if pr % 2 == 0:
                    nc.scalar.copy(out=dst, in_=po[:])
                else:
                    nc.vector.tensor_copy(out=dst, in_=po[:])
                pair += 1
                if i == 0 and gi == 0:
                    # First window: per-pair DMAs start the write phase
                    # earlier.
                    nc.sync.dma_start(out_d[i][:, lo:lo + 1024],
                                      orow[:, lo:lo + 1024])
                elif pr % 2 == 1:
                    nc.sync.dma_start(out_d[i][:, 2048 * gi:2048 * (gi + 1)],
                                      orow[:, 2048 * gi:2048 * (gi + 1)])
    nc.compile()
    return nc


def kernel(traj_loc, mat2, vec, traj_len, l_max, emb_sl_w, emb_su_w,
           emb_tl_w, emb_tu_w):
    from concourse import bass_utils

    traj_loc = np.asarray(traj_loc).astype(np.int64)
    mat2 = np.ascontiguousarray(np.asarray(mat2, dtype=np.float32))
    traj_len = np.asarray(traj_len).astype(np.int64)
    esl = np.asarray(emb_sl_w, dtype=np.float32)
    esu = np.asarray(emb_su_w, dtype=np.float32)
    etl = np.asarray(emb_tl_w, dtype=np.float32)
    etu = np.asarray(emb_tu_w, dtype=np.float32)

    # host prep: constants
    A = esl + etl                                            # [2, E]
    B = (esu - esl) / np.float32(SU) + (etu - etl) / np.float32(TU)
    mask = (np.arange(M)[None, :] < traj_len[:, None])       # [N, M]
    idx_full = np.where(mask, traj_loc - 1, NLOC).astype(np.int32)

    mat2x = np.concatenate([mat2, np.zeros((1, L), np.float32)], axis=0)
    m2 = mat2x.astype(np.float16)
    b1 = B[1].astype(np.float16)
    dA = (A[1] - A[0]).astype(np.float16)
    a0 = A[0].astype(np.float16)

    # rhs[s] is [34, 8E]: row 8*s+lp selects l' = lp within the window and
    # scales e-block lp by b1; rows 32-33 pair with lhsT rows [m, 1]:
    # m*dA + a0, replicated across all 8 e-blocks.
    rhs = np.zeros((4, K, 8 * E), np.float16)
    for s in range(4):
        for lp in range(8):
            rhs[s, 8 * s + lp, E * lp:E * (lp + 1)] = b1
        rhs[s, 32, :] = np.tile(dA, 8)
        rhs[s, 33, :] = np.tile(a0, 8)
    ident = np.eye(128, dtype=np.float16)

    # mrow[i] = [m, 1] rows for lhsT rows 32-33, tiled 4x along the free
    # dim so one DMA fills all four gi windows of a row's lhsT tile.
    mrow_full = np.empty((N, 2, 4 * M), np.float16)
    mrow_full[:, 0, :] = np.tile(mask.astype(np.float16), (1, 4))
    mrow_full[:, 1, :] = np.ones((1, 4 * M), np.float16)

    if "nc" not in _CACHE:
        _CACHE["nc"] = _build()
    nc = _CACHE["nc"]

    in_maps = []
    for c in range(NCORES):
        sl = slice(ROWS * c, ROWS * (c + 1))
        in_maps.append({
            "m2": m2,
            "idx": np.ascontiguousarray(idx_full[sl].T),
            "mrow": np.ascontiguousarray(mrow_full[sl]),
            "rhs": rhs,
            "ident": ident,
        })

    trace = os.environ.get("KERNEL_TRACE", "0") == "1" and _install_profhook()
    res = bass_utils.run_bass_kernel_spmd(
        nc, in_maps, core_ids=list(range(NCORES)), trace=bool(trace))
    if trace:
        _CACHE["exec_time_ns"] = res.exec_time_ns
        _CACHE["trace_path"] = (res.instructions_and_trace or (None, None))[1]
        _CACHE["tmpdir"] = res.profile_json

    out = np.concatenate(
        [res.results[c]["out"].reshape(ROWS, M, L, E) for c in range(NCORES)],
        axis=0)
    return out.astype(np.float32)
